# revision 9
# baseline (speedup 1.0000x reference)
"""MultiHeadAttention forward on 8 Trainium2 NeuronCores (v2).

Problem: B=2, S=2048, D_MODEL=1024, H=16 heads, d_k=64, causal mask.

Sharding v2: core c -> (batch b = c//4, head-group hg = c%4). Each core
computes attention for heads {4hg..4hg+3} of batch b, so it only loads
batch b's activations (12 MB instead of 24 MB per core).

 - Projections: Q^T,K^T per head-pair p in {0,1} as [128, S] tiles
   (rows: head 2p d_k 0-63, head 2p+1 64-127); V as packed [128, 130]
   tiles per 128-kpos block (cols 0:64 head even | col 64 ones |
   65:129 head odd | col 129 ones) -- the ones columns produce softmax
   denominators inside the attn@V matmuls.
 - Attention per pair, q-blocks of 512, kt blocks of 128 kpos.
   ScoresT[kpos, q] via two row-group-concurrent matmuls (K=64 each).
   exp on ScalarE (scale 1/8 folded). Diagonal kt tiles restrict all
   work (scores / exp / attnV) to the live columns [128t, 512) and
   multiply only the [128,128] triangle block by a causal mask.
 - Normalization: evacuate av psum [65, 1024], reciprocal_approx_fast
   on the denominator row, gpsimd partition-broadcast, multiply into
   AFT (attn_flatT, bf16).
 - Output: per head-pair AllToAll over the 4 cores of the same batch
   redistributes AFT so core j gets d-rows {256i+128p} for its q-slice
   [512j, 512j+512). Pair-0's collective + half of the W_o matmul
   overlap pair-1's attention. out = lhs.T @ W_o + b_o.

Matmuls in bf16 (host casts); fp32 accumulation in PSUM.
"""

import sys

import numpy as np

sys.path.insert(0, "/opt/trn_rl_repo")

import ml_dtypes  # noqa: E402

import concourse.bacc as bacc  # noqa: E402
import concourse.mybir as mybir  # noqa: E402
import concourse.tile as tile  # noqa: E402
from concourse.bass_utils import run_bass_kernel_spmd  # noqa: E402

F32 = mybir.dt.float32
BF16 = mybir.dt.bfloat16
BF = ml_dtypes.bfloat16

B, S, D, H, DK = 2, 2048, 1024, 16, 64
N_CORES = 8
HPC = 4  # heads per core
DPC = HPC * DK  # 256 d_model cols per core
NKT = S // 128  # 16 kpos tiles
NQB = S // 512  # 4 q blocks

_CACHED = {}


def build_nc():
    nc = bacc.Bacc(num_devices=N_CORES)

    # ---- I/O (per core: batch b = c//4, cols 256*hg..) ----
    # x stored quarter-major, p-major within quarter: [4, 128, 8*512]
    # (element [q, p, 512*t + c] = x[b].T[128*t + p, 512*q + c])
    xq = nc.dram_tensor("xq", [4, 128, 4096], BF16, kind="ExternalInput")
    xk = nc.dram_tensor("xk", [4, 128, 4096], BF16, kind="ExternalInput")
    xv = nc.dram_tensor("xv", [4, 128, 4096], BF16, kind="ExternalInput")
    # w p-major: [128, 8*256], element [p, 256*t + c] = W[128*t + p, c]
    wq = nc.dram_tensor("wq", [128, 8 * DPC], BF16, kind="ExternalInput")
    wk = nc.dram_tensor("wk", [128, 8 * DPC], BF16, kind="ExternalInput")
    wv = nc.dram_tensor("wv", [128, 8 * DPC], BF16, kind="ExternalInput")
    wo = nc.dram_tensor("wo", [D, D], BF16, kind="ExternalInput")
    bq = nc.dram_tensor("bq", [128, 2], F32, kind="ExternalInput")
    bk = nc.dram_tensor("bk", [128, 2], F32, kind="ExternalInput")
    bv = nc.dram_tensor("bv", [1, DPC], F32, kind="ExternalInput")
    bo = nc.dram_tensor("bo", [1, D], F32, kind="ExternalInput")
    tri = nc.dram_tensor("tri", [128, 128], BF16, kind="ExternalInput")
    qoff = nc.dram_tensor("qoff", [1, 1], mybir.dt.int32, kind="ExternalInput")
    out = nc.dram_tensor("out", [512, D], F32, kind="ExternalOutput")

    with tile.TileContext(nc) as tc:
        with (
            tc.tile_pool(name="wtiles", bufs=1) as w_pool,
            tc.tile_pool(name="persist", bufs=1) as persist,
            tc.tile_pool(name="exp", bufs=4) as exp_pool,
            tc.tile_pool(name="small", bufs=2) as small_pool,
            tc.tile_pool(name="gen_ps", bufs=2, space="PSUM") as gen_ps,
            tc.tile_pool(name="score_ps", bufs=2, space="PSUM") as score_ps,
            tc.tile_pool(name="av_ps", bufs=1, space="PSUM") as av_ps,
            tc.tile_pool(name="dram", bufs=1, space="DRAM") as dram,
        ):
            # ---- persistent SBUF tensors ----
            QT = [persist.tile([128, S], BF16, tag=f"QT{p}", name=f"QT{p}") for p in range(2)]
            KT = [persist.tile([128, S], BF16, tag=f"KT{p}", name=f"KT{p}") for p in range(2)]
            VP = [
                [persist.tile([128, 130], BF16, tag=f"VP{p}_{i}", name=f"VP{p}_{i}") for i in range(NKT)]
                for p in range(2)
            ]
            AFT = [persist.tile([128, S], BF16, tag=f"AFT{p}", name=f"AFT{p}") for p in range(2)]
            tri_t = persist.tile([128, 128], BF16, tag="tri")
            nc.scalar.dma_start(tri_t[:], tri[:])

            bq_t = persist.tile([128, 2], F32, tag="bq")
            bk_t = persist.tile([128, 2], F32, tag="bk")
            nc.scalar.dma_start(bq_t[:], bq[:])
            nc.scalar.dma_start(bk_t[:], bk[:])
            bv_bc = persist.tile([128, DPC], F32, tag="bvbc")
            nc.scalar.dma_start(bv_bc[:], bv[:].partition_broadcast(128))
            bo_bc = persist.tile([128, D], F32, tag="bobc")
            nc.scalar.dma_start(bo_bc[:], bo[:].partition_broadcast(128))

            # packed weight tiles: [128, 8, 256] (d-tile t at [:, t, :]); one DMA each
            wq_a = w_pool.tile([128, 8, DPC], BF16, tag="wq_a")
            wk_a = w_pool.tile([128, 8, DPC], BF16, tag="wk_a")
            wv_a = w_pool.tile([128, 8, DPC], BF16, tag="wv_a")
            nc.scalar.dma_start(wq_a[:], wq[:])
            nc.scalar.dma_start(wk_a[:], wk[:])
            nc.scalar.dma_start(wv_a[:], wv[:])
            wq_t = [wq_a[:, d, :] for d in range(8)]
            wk_t = [wk_a[:, d, :] for d in range(8)]
            wv_t = [wv_a[:, d, :] for d in range(8)]
            wo_t = [w_pool.tile([128, D], BF16, tag=f"wo{d}", name=f"wo{d}") for d in range(8)]

            # ones columns of the packed V tiles (written once)
            for p in range(2):
                for i in range(NKT):
                    ones_ap = VP[p][i][:, 0:130].rearrange("a (h c) -> a h c", h=2)[:, :, 64:65]
                    nc.gpsimd.memset(ones_ap, 1.0)

            # ---- helpers ----
            def proj_qk_sc(p, sc, xa, wt, bias_t, dst):
                ps = gen_ps.tile([128, 512], F32, tag="gen", name="gen")
                for d in range(8):
                    nc.tensor.matmul(
                        ps[:],
                        wt[d][:, 128 * p : 128 * (p + 1)],
                        xa[:, sc, d, :],
                        start=(d == 0),
                        stop=(d == 7),
                    )
                nc.vector.tensor_scalar_add(
                    dst[:, 512 * sc : 512 * (sc + 1)], ps[:], bias_t[:, p : p + 1]
                )

            def proj_qk(p, xa, wt, bias_t, dst):
                for sc in range(4):
                    proj_qk_sc(p, sc, xa, wt, bias_t, dst)

            def proj_v_sc(sc, xa):
                for ss in range(4 * sc, 4 * sc + 4):
                    k = ss % 4
                    ps = gen_ps.tile([128, 512], F32, tag="gen", name="gen")
                    for d in range(8):
                        nc.tensor.matmul(
                            ps[:, 0:DPC],
                            xa[:, sc, d, 128 * k : 128 * (k + 1)],
                            wv_t[d],
                            start=(d == 0),
                            stop=(d == 7),
                        )
                    for p in range(2):
                        dst = VP[p][ss][:, 0:130].rearrange("a (h c) -> a h c", h=2)[:, :, 0:64]
                        src = ps[:, 128 * p : 128 * (p + 1)].rearrange("a (h c) -> a h c", h=2)
                        bsl = bv_bc[:, 128 * p : 128 * (p + 1)].rearrange("a (h c) -> a h c", h=2)
                        nc.vector.tensor_add(dst, src, bsl)

            def attn_qblock(p, qb):
                qcol = 512 * qb
                n_kt = 4 * qb + 4
                av = av_ps.tile([65, 1024], F32, tag="av")

                def emit_scores(kt):
                    t = kt - 4 * qb
                    c0 = 128 * t if t >= 0 else 0
                    ps = score_ps.tile([128, 1024], F32, tag="sc", name="sc")
                    ksl = slice(128 * kt, 128 * (kt + 1))
                    for h in range(2):
                        nc.tensor.matmul(
                            ps[:, 512 * h + c0 : 512 * (h + 1)],
                            KT[p][64 * h : 64 * (h + 1), ksl],
                            QT[p][64 * h : 64 * (h + 1), qcol + c0 : qcol + 512],
                            start=True,
                            stop=True,
                        )
                    return ps

                ps_cur = emit_scores(0)
                for kt in range(n_kt):
                    t = kt - 4 * qb
                    c0 = 128 * t if t >= 0 else 0
                    et = exp_pool.tile([128, 1024], BF16, tag="et")
                    if c0 == 0:
                        nc.scalar.activation(
                            et[:], ps_cur[:], mybir.ActivationFunctionType.Exp, scale=0.125
                        )
                    else:
                        e3 = et[:, 0:1024].rearrange("a (h q) -> a h q", h=2)[:, :, c0:512]
                        p3 = ps_cur[:, 0:1024].rearrange("a (h q) -> a h q", h=2)[:, :, c0:512]
                        nc.scalar.activation(
                            e3, p3, mybir.ActivationFunctionType.Exp, scale=0.125
                        )
                    if kt + 1 < n_kt:
                        ps_cur = emit_scores(kt + 1)
                    if t >= 0:
                        for h in range(2):
                            dsl = slice(512 * h + c0, 512 * h + c0 + 128)
                            nc.vector.tensor_mul(et[:, dsl], et[:, dsl], tri_t[:])
                    for h in range(2):
                        nc.tensor.matmul(
                            av[:, 512 * h + c0 : 512 * (h + 1)],
                            VP[p][kt][:, 65 * h : 65 * (h + 1)],
                            et[:, 512 * h + c0 : 512 * (h + 1)],
                            start=(kt == 0),
                            stop=(kt == n_kt - 1),
                        )
                # normalize: evacuate psum, 1/denom, broadcast, scale into AFT
                avs = small_pool.tile([64, 1024], F32, tag="avs", name="avs")
                nc.vector.tensor_copy(avs[:], av[0:64, :])
                dn = small_pool.tile([1, 1024], F32, tag="dn")
                nc.vector.tensor_copy(dn[:], av[64:65, :])
                rcp = small_pool.tile([1, 1024], F32, tag="rcp")
                nc.vector.reciprocal_approx_fast(rcp[:], dn[:])
                rbc = small_pool.tile([64, 1024], F32, tag="rbc")
                nc.gpsimd.partition_broadcast(rbc[:], rcp[:])
                for h in range(2):
                    nc.vector.tensor_mul(
                        AFT[p][64 * h : 64 * (h + 1), qcol : qcol + 512],
                        avs[0:64, 512 * h : 512 * (h + 1)],
                        rbc[:, 512 * h : 512 * (h + 1)],
                    )
                # stage this q-block's columns for the AllToAll (dests qb, qb+4)
                for j in (qb, qb + 4):
                    nc.sync.dma_start(
                        a2a_in[p][128 * j : 128 * (j + 1), :],
                        AFT[p][:, qcol : qcol + 512],
                    )

            a2a_in = [dram.tile([1024, 512], BF16, tag=f"a2a_in{p}", name=f"a2a_in{p}") for p in range(2)]
            a2a_out = [dram.tile([2, 512, 512], BF16, tag=f"a2a_out{p}", name=f"a2a_out{p}") for p in range(2)]

            # ---- load x, project, attention pair 0 (pair-1 proj interleaved) ----
            with (
                tc.tile_pool(name="xtq", bufs=1) as xq_pool,
                tc.tile_pool(name="xtk", bufs=1) as xk_pool,
                tc.tile_pool(name="xtv", bufs=1) as xv_pool,
            ):
                xk_a = xk_pool.tile([128, 4, 8, 512], BF16, tag="xk_a")
                xv_a = xv_pool.tile([128, 4, 8, 512], BF16, tag="xv_a")
                xq_a = xq_pool.tile([128, 4, 8, 512], BF16, tag="xq_a")
                # quarter loads: one fully-contiguous [128 x 8KB] DMA each, so
                # pair-0 projections and attention q-block 0 start early
                for sc in range(4):
                    for xa_, xd in ((xk_a, xk), (xv_a, xv), (xq_a, xq)):
                        nc.sync.dma_start(xa_[:, sc, :, :], xd[sc, :, :])
                    proj_qk_sc(0, sc, xk_a, wk_t, bk_t, KT[0])
                    proj_v_sc(sc, xv_a)
                    proj_qk_sc(0, sc, xq_a, wq_t, bq_t, QT[0])

                # W_o loads after x so x DMAs get the early bandwidth
                for d in range(8):
                    nc.sync.dma_start(wo_t[d][:], wo[128 * d : 128 * (d + 1), :])

                attn_qblock(0, 0)
                proj_qk(1, xk_a, wk_t, bk_t, KT[1])
                attn_qblock(0, 1)
                proj_qk(1, xq_a, wq_t, bq_t, QT[1])
                attn_qblock(0, 2)
                attn_qblock(0, 3)

            with tc.tile_pool(name="late", bufs=1) as late:
                lhs_t = [
                    [late.tile([128, 512], BF16, tag=f"lhs{p}_{i}", name=f"lhs{p}_{i}") for i in range(4)]
                    for p in range(2)
                ]
                osb = [late.tile([128, D], F32, tag=f"osb{st}", name=f"osb{st}") for st in range(4)]

                # my batch-group (core // 4), from a per-core input
                import concourse.bass as bass_mod
                gsel_reg = nc.alloc_registers("gsel_reg")
                nc.regs_load(gsel_reg, qoff[0:1, 0:1])
                gsel_s = nc.snap(gsel_reg, donate=True, min_val=0, max_val=1)

                def emit_a2a(p):
                    nc.gpsimd.collective_compute(
                        "AllToAll",
                        mybir.AluOpType.bypass,
                        replica_groups=[list(range(8))],
                        ins=[a2a_in[p][:]],
                        outs=[a2a_out[p][:]],
                    )
                    for i in range(4):
                        nc.sync.dma_start(
                            lhs_t[p][i][:],
                            a2a_out[p][bass_mod.ds(gsel_s, 1), 128 * i : 128 * (i + 1), :],
                        )

                def emit_wo(p):
                    for st in range(4):
                        for nch in range(2):
                            ps = gen_ps.tile([128, 512], F32, tag="gen")
                            for i in range(4):
                                nc.tensor.matmul(
                                    ps[:],
                                    lhs_t[p][i][:, 128 * st : 128 * (st + 1)],
                                    wo_t[2 * i + p][:, 512 * nch : 512 * (nch + 1)],
                                    start=(i == 0),
                                    stop=(i == 3),
                                )
                            osl = slice(512 * nch, 512 * (nch + 1))
                            if p == 0:
                                nc.vector.tensor_add(osb[st][:, osl], ps[:], bo_bc[:, osl])
                            else:
                                nc.vector.tensor_add(osb[st][:, osl], ps[:], osb[st][:, osl])
                        if p == 1:
                            nc.sync.dma_start(out[128 * st : 128 * (st + 1), :], osb[st][:])

                emit_a2a(0)
                attn_qblock(1, 0)
                attn_qblock(1, 1)
                attn_qblock(1, 2)
                attn_qblock(1, 3)
                emit_wo(0)
                emit_a2a(1)
                emit_wo(1)

    nc.finalize()
    return nc


def _prep_in_maps(q, k, v, W_q, b_q, W_k, b_k, W_v, b_v, W_o, b_o):
    def qpmajor(x):  # [S, D] -> [4, 128, 8*512]: [q, p, 512*t + c] = xT[128t+p, 512q+c]
        xt = x.T.astype(BF)  # [D, S]
        return np.ascontiguousarray(
            xt.reshape(8, 128, 4, 512).transpose(2, 1, 0, 3).reshape(4, 128, 4096)
        )

    xT = [(qpmajor(q[b]), qpmajor(k[b]), qpmajor(v[b])) for b in range(B)]
    wo_h = np.ascontiguousarray(W_o.astype(BF))
    bo_h = np.ascontiguousarray(b_o.reshape(1, D).astype(np.float32))
    i = np.arange(128)
    tri_h = np.ascontiguousarray((i[:, None] <= i[None, :]).astype(BF))

    def pmajor_w(w):  # [1024, 256] -> [128, 8*256]: [p, 256t + c] = w[128t+p, c]
        return np.ascontiguousarray(
            w.astype(BF).reshape(8, 128, DPC).transpose(1, 0, 2).reshape(128, 8 * DPC)
        )

    in_maps = []
    for c in range(N_CORES):
        b, hg = c // 4, c % 4
        csl = slice(DPC * hg, DPC * (hg + 1))
        in_maps.append(
            {
                "xq": xT[b][0],
                "xk": xT[b][1],
                "xv": xT[b][2],
                "wq": pmajor_w(W_q[:, csl]),
                "wk": pmajor_w(W_k[:, csl]),
                "wv": pmajor_w(W_v[:, csl]),
                "wo": wo_h,
                "bq": np.ascontiguousarray(
                    b_q[csl].reshape(2, 128).T.astype(np.float32)
                ),
                "bk": np.ascontiguousarray(
                    b_k[csl].reshape(2, 128).T.astype(np.float32)
                ),
                "bv": np.ascontiguousarray(b_v[csl].reshape(1, DPC).astype(np.float32)),
                "bo": bo_h,
                "tri": tri_h,
                "qoff": np.array([[b]], dtype=np.int32),
            }
        )
    return in_maps


def kernel(q, k, v, mask, W_q, b_q, W_k, b_k, W_v, b_v, W_o, b_o, **run_kwargs):
    q, k, v = (np.asarray(t, np.float32) for t in (q, k, v))
    in_maps = _prep_in_maps(
        q, k, v,
        np.asarray(W_q, np.float32), np.asarray(b_q, np.float32),
        np.asarray(W_k, np.float32), np.asarray(b_k, np.float32),
        np.asarray(W_v, np.float32), np.asarray(b_v, np.float32),
        np.asarray(W_o, np.float32), np.asarray(b_o, np.float32),
    )
    if "nc" not in _CACHED:
        _CACHED["nc"] = build_nc()
    res = run_bass_kernel_spmd(
        _CACHED["nc"], in_maps, core_ids=list(range(N_CORES)), **run_kwargs
    )
    _CACHED["last_result"] = res
    full = np.empty((B, S, D), np.float32)
    for c in range(N_CORES):
        b, hg = c // 4, c % 4
        full[b, 512 * hg : 512 * (hg + 1), :] = res.results[c]["out"]
    return full


if __name__ == "__main__":
    build_nc()
    print("build ok")


# revision 10
# speedup vs baseline: 1.0771x; 1.0771x over previous
"""MultiHeadAttention forward on 8 Trainium2 NeuronCores (v2).

Problem: B=2, S=2048, D_MODEL=1024, H=16 heads, d_k=64, causal mask.

Sharding v2: core c -> (batch b = c//4, head-group hg = c%4). Each core
computes attention for heads {4hg..4hg+3} of batch b, so it only loads
batch b's activations (12 MB instead of 24 MB per core).

 - Projections: Q^T,K^T per head-pair p in {0,1} as [128, S] tiles
   (rows: head 2p d_k 0-63, head 2p+1 64-127); V as packed [128, 130]
   tiles per 128-kpos block (cols 0:64 head even | col 64 ones |
   65:129 head odd | col 129 ones) -- the ones columns produce softmax
   denominators inside the attn@V matmuls.
 - Attention per pair, q-blocks of 512, kt blocks of 128 kpos.
   ScoresT[kpos, q] via two row-group-concurrent matmuls (K=64 each).
   exp on ScalarE (scale 1/8 folded). Diagonal kt tiles restrict all
   work (scores / exp / attnV) to the live columns [128t, 512) and
   multiply only the [128,128] triangle block by a causal mask.
 - Normalization: evacuate av psum [65, 1024], reciprocal_approx_fast
   on the denominator row, gpsimd partition-broadcast, multiply into
   AFT (attn_flatT, bf16).
 - Output: per head-pair AllToAll over the 4 cores of the same batch
   redistributes AFT so core j gets d-rows {256i+128p} for its q-slice
   [512j, 512j+512). Pair-0's collective + half of the W_o matmul
   overlap pair-1's attention. out = lhs.T @ W_o + b_o.

Matmuls in bf16 (host casts); fp32 accumulation in PSUM.
"""

import sys

import numpy as np

sys.path.insert(0, "/opt/trn_rl_repo")

import ml_dtypes  # noqa: E402

import concourse.bacc as bacc  # noqa: E402
import concourse.mybir as mybir  # noqa: E402
import concourse.tile as tile  # noqa: E402
from concourse.bass_utils import run_bass_kernel_spmd  # noqa: E402

F32 = mybir.dt.float32
BF16 = mybir.dt.bfloat16
BF = ml_dtypes.bfloat16

B, S, D, H, DK = 2, 2048, 1024, 16, 64
N_CORES = 8
HPC = 4  # heads per core
DPC = HPC * DK  # 256 d_model cols per core
NKT = S // 128  # 16 kpos tiles
NQB = S // 512  # 4 q blocks

_CACHED = {}


def build_nc():
    nc = bacc.Bacc(num_devices=N_CORES)

    # ---- I/O (per core: batch b = c//4, cols 256*hg..) ----
    # x stored quarter-major, p-major within quarter: [4, 128, 8*512]
    # (element [q, p, 512*t + c] = x[b].T[128*t + p, 512*q + c])
    xq = nc.dram_tensor("xq", [4, 128, 4096], BF16, kind="ExternalInput")
    xk = nc.dram_tensor("xk", [4, 128, 4096], BF16, kind="ExternalInput")
    xv = nc.dram_tensor("xv", [4, 128, 4096], BF16, kind="ExternalInput")
    # w p-major: [128, 8*256], element [p, 256*t + c] = W[128*t + p, c]
    wq = nc.dram_tensor("wq", [128, 8 * DPC], BF16, kind="ExternalInput")
    wk = nc.dram_tensor("wk", [128, 8 * DPC], BF16, kind="ExternalInput")
    wv = nc.dram_tensor("wv", [128, 8 * DPC], BF16, kind="ExternalInput")
    wo = nc.dram_tensor("wo", [D, D], BF16, kind="ExternalInput")
    bq = nc.dram_tensor("bq", [128, 2], F32, kind="ExternalInput")
    bk = nc.dram_tensor("bk", [128, 2], F32, kind="ExternalInput")
    bv = nc.dram_tensor("bv", [1, DPC], F32, kind="ExternalInput")
    bo = nc.dram_tensor("bo", [1, D], F32, kind="ExternalInput")
    tri = nc.dram_tensor("tri", [128, 128], BF16, kind="ExternalInput")
    qoff = nc.dram_tensor("qoff", [1, 1], mybir.dt.int32, kind="ExternalInput")
    out = nc.dram_tensor("out", [512, D], F32, kind="ExternalOutput")

    with tile.TileContext(nc) as tc:
        with (
            tc.tile_pool(name="wtiles", bufs=1) as w_pool,
            tc.tile_pool(name="persist", bufs=1) as persist,
            tc.tile_pool(name="exp", bufs=4) as exp_pool,
            tc.tile_pool(name="small", bufs=2) as small_pool,
            tc.tile_pool(name="gen_ps", bufs=2, space="PSUM") as gen_ps,
            tc.tile_pool(name="score_ps", bufs=2, space="PSUM") as score_ps,
            tc.tile_pool(name="av_ps", bufs=1, space="PSUM") as av_ps,
            tc.tile_pool(name="dram", bufs=1, space="DRAM") as dram,
        ):
            # ---- persistent SBUF tensors ----
            QT = [persist.tile([128, S], BF16, tag=f"QT{p}", name=f"QT{p}") for p in range(2)]
            KT = [persist.tile([128, S], BF16, tag=f"KT{p}", name=f"KT{p}") for p in range(2)]
            VP = [
                [persist.tile([128, 130], BF16, tag=f"VP{p}_{i}", name=f"VP{p}_{i}") for i in range(NKT)]
                for p in range(2)
            ]
            AFT = [persist.tile([128, S], BF16, tag=f"AFT{p}", name=f"AFT{p}") for p in range(2)]
            tri_t = persist.tile([128, 128], BF16, tag="tri")
            nc.scalar.dma_start(tri_t[:], tri[:])

            bq_t = persist.tile([128, 2], F32, tag="bq")
            bk_t = persist.tile([128, 2], F32, tag="bk")
            nc.scalar.dma_start(bq_t[:], bq[:])
            nc.scalar.dma_start(bk_t[:], bk[:])
            bv_bc = persist.tile([128, DPC], F32, tag="bvbc")
            nc.scalar.dma_start(bv_bc[:], bv[:].partition_broadcast(128))
            bo_bc = persist.tile([128, D], F32, tag="bobc")
            nc.scalar.dma_start(bo_bc[:], bo[:].partition_broadcast(128))

            # packed weight tiles: [128, 8, 256] (d-tile t at [:, t, :]); one DMA each
            wq_a = w_pool.tile([128, 8, DPC], BF16, tag="wq_a")
            wk_a = w_pool.tile([128, 8, DPC], BF16, tag="wk_a")
            wv_a = w_pool.tile([128, 8, DPC], BF16, tag="wv_a")
            nc.scalar.dma_start(wq_a[:], wq[:])
            nc.scalar.dma_start(wk_a[:], wk[:])
            nc.scalar.dma_start(wv_a[:], wv[:])
            wq_t = [wq_a[:, d, :] for d in range(8)]
            wk_t = [wk_a[:, d, :] for d in range(8)]
            wv_t = [wv_a[:, d, :] for d in range(8)]
            wo_t = [w_pool.tile([128, D], BF16, tag=f"wo{d}", name=f"wo{d}") for d in range(8)]

            # ones columns of the packed V tiles (written once)
            for p in range(2):
                for i in range(NKT):
                    ones_ap = VP[p][i][:, 0:130].rearrange("a (h c) -> a h c", h=2)[:, :, 64:65]
                    nc.gpsimd.memset(ones_ap, 1.0)

            # ---- helpers ----
            def proj_qk_sc(p, sc, xa, wt, bias_t, dst):
                ps = gen_ps.tile([128, 512], F32, tag="gen", name="gen")
                for d in range(8):
                    nc.tensor.matmul(
                        ps[:],
                        wt[d][:, 128 * p : 128 * (p + 1)],
                        xa[:, sc, d, :],
                        start=(d == 0),
                        stop=(d == 7),
                    )
                nc.vector.tensor_scalar_add(
                    dst[:, 512 * sc : 512 * (sc + 1)], ps[:], bias_t[:, p : p + 1]
                )

            def proj_qk(p, xa, wt, bias_t, dst):
                for sc in range(4):
                    proj_qk_sc(p, sc, xa, wt, bias_t, dst)

            def proj_v_sc(sc, xa):
                for ss in range(4 * sc, 4 * sc + 4):
                    k = ss % 4
                    ps = gen_ps.tile([128, 512], F32, tag="gen", name="gen")
                    for d in range(8):
                        nc.tensor.matmul(
                            ps[:, 0:DPC],
                            xa[:, sc, d, 128 * k : 128 * (k + 1)],
                            wv_t[d],
                            start=(d == 0),
                            stop=(d == 7),
                        )
                    for p in range(2):
                        dst = VP[p][ss][:, 0:130].rearrange("a (h c) -> a h c", h=2)[:, :, 0:64]
                        src = ps[:, 128 * p : 128 * (p + 1)].rearrange("a (h c) -> a h c", h=2)
                        bsl = bv_bc[:, 128 * p : 128 * (p + 1)].rearrange("a (h c) -> a h c", h=2)
                        nc.vector.tensor_add(dst, src, bsl)

            def attn_qblock(p, qb):
                qcol = 512 * qb
                n_kt = 4 * qb + 4
                av = av_ps.tile([65, 1024], F32, tag="av")

                def emit_scores(kt):
                    t = kt - 4 * qb
                    c0 = 128 * t if t >= 0 else 0
                    ps = score_ps.tile([128, 1024], F32, tag="sc", name="sc")
                    ksl = slice(128 * kt, 128 * (kt + 1))
                    for h in range(2):
                        nc.tensor.matmul(
                            ps[:, 512 * h + c0 : 512 * (h + 1)],
                            KT[p][64 * h : 64 * (h + 1), ksl],
                            QT[p][64 * h : 64 * (h + 1), qcol + c0 : qcol + 512],
                            start=True,
                            stop=True,
                        )
                    return ps

                ps_cur = emit_scores(0)
                for kt in range(n_kt):
                    t = kt - 4 * qb
                    c0 = 128 * t if t >= 0 else 0
                    et = exp_pool.tile([128, 1024], BF16, tag="et")
                    if c0 == 0:
                        nc.scalar.activation(
                            et[:], ps_cur[:], mybir.ActivationFunctionType.Exp, scale=0.125
                        )
                    else:
                        e3 = et[:, 0:1024].rearrange("a (h q) -> a h q", h=2)[:, :, c0:512]
                        p3 = ps_cur[:, 0:1024].rearrange("a (h q) -> a h q", h=2)[:, :, c0:512]
                        nc.scalar.activation(
                            e3, p3, mybir.ActivationFunctionType.Exp, scale=0.125
                        )
                    if kt + 1 < n_kt:
                        ps_cur = emit_scores(kt + 1)
                    if t >= 0:
                        for h in range(2):
                            dsl = slice(512 * h + c0, 512 * h + c0 + 128)
                            nc.vector.tensor_mul(et[:, dsl], et[:, dsl], tri_t[:])
                    for h in range(2):
                        nc.tensor.matmul(
                            av[:, 512 * h + c0 : 512 * (h + 1)],
                            VP[p][kt][:, 65 * h : 65 * (h + 1)],
                            et[:, 512 * h + c0 : 512 * (h + 1)],
                            start=(kt == 0),
                            stop=(kt == n_kt - 1),
                        )
                # normalize: evacuate psum, 1/denom, broadcast, scale into AFT
                avs = small_pool.tile([64, 1024], F32, tag="avs", name="avs")
                nc.vector.tensor_copy(avs[:], av[0:64, :])
                dn = small_pool.tile([1, 1024], F32, tag="dn")
                nc.vector.tensor_copy(dn[:], av[64:65, :])
                rcp = small_pool.tile([1, 1024], F32, tag="rcp")
                nc.vector.reciprocal_approx_fast(rcp[:], dn[:])
                rbc = small_pool.tile([64, 1024], F32, tag="rbc")
                nc.gpsimd.partition_broadcast(rbc[:], rcp[:])
                for h in range(2):
                    nc.vector.tensor_mul(
                        AFT[p][64 * h : 64 * (h + 1), qcol : qcol + 512],
                        avs[0:64, 512 * h : 512 * (h + 1)],
                        rbc[:, 512 * h : 512 * (h + 1)],
                    )
                # stage this q-block's columns for the AllToAll (dests qb, qb+4)
                for j in (qb, qb + 4):
                    nc.sync.dma_start(
                        a2a_in[p][128 * j : 128 * (j + 1), :],
                        AFT[p][:, qcol : qcol + 512],
                    )

            a2a_in = [dram.tile([1024, 512], BF16, tag=f"a2a_in{p}", name=f"a2a_in{p}") for p in range(2)]
            a2a_out = [dram.tile([2, 512, 512], BF16, tag=f"a2a_out{p}", name=f"a2a_out{p}") for p in range(2)]

            # ---- load x, project, attention pair 0 (pair-1 proj interleaved) ----
            with (
                tc.tile_pool(name="xtq", bufs=1) as xq_pool,
                tc.tile_pool(name="xtk", bufs=1) as xk_pool,
                tc.tile_pool(name="xtv", bufs=1) as xv_pool,
            ):
                xk_a = xk_pool.tile([128, 4, 8, 512], BF16, tag="xk_a")
                xv_a = xv_pool.tile([128, 4, 8, 512], BF16, tag="xv_a")
                xq_a = xq_pool.tile([128, 4, 8, 512], BF16, tag="xq_a")
                # quarter loads: one fully-contiguous [128 x 8KB] DMA each, so
                # pair-0 projections and attention q-block 0 start early
                for sc in range(4):
                    for xa_, xd in ((xk_a, xk), (xv_a, xv), (xq_a, xq)):
                        nc.sync.dma_start(xa_[:, sc, :, :], xd[sc, :, :])
                    proj_qk_sc(0, sc, xk_a, wk_t, bk_t, KT[0])
                    proj_v_sc(sc, xv_a)
                    proj_qk_sc(0, sc, xq_a, wq_t, bq_t, QT[0])

                attn_qblock(0, 0)
                # W_o loads late: needed only by the post-attention Wo matmuls
                for d in range(8):
                    nc.sync.dma_start(wo_t[d][:], wo[128 * d : 128 * (d + 1), :])
                proj_qk(1, xk_a, wk_t, bk_t, KT[1])
                attn_qblock(0, 1)
                proj_qk(1, xq_a, wq_t, bq_t, QT[1])
                attn_qblock(0, 2)
                attn_qblock(0, 3)

            with tc.tile_pool(name="late", bufs=1) as late:
                lhs_t = [
                    [late.tile([128, 512], BF16, tag=f"lhs{p}_{i}", name=f"lhs{p}_{i}") for i in range(4)]
                    for p in range(2)
                ]
                osb = [late.tile([128, D], F32, tag=f"osb{st}", name=f"osb{st}") for st in range(4)]

                # my batch-group (core // 4), from a per-core input
                import concourse.bass as bass_mod
                gsel_reg = nc.alloc_registers("gsel_reg")
                nc.regs_load(gsel_reg, qoff[0:1, 0:1])
                gsel_s = nc.snap(gsel_reg, donate=True, min_val=0, max_val=1)

                def emit_a2a_trigger(p):
                    nc.gpsimd.collective_compute(
                        "AllToAll",
                        mybir.AluOpType.bypass,
                        replica_groups=[list(range(8))],
                        ins=[a2a_in[p][:]],
                        outs=[a2a_out[p][:]],
                    )

                def emit_readback(p):
                    for i in range(4):
                        nc.sync.dma_start(
                            lhs_t[p][i][:],
                            a2a_out[p][bass_mod.ds(gsel_s, 1), 128 * i : 128 * (i + 1), :],
                        )

                def emit_wo(p):
                    for st in range(4):
                        for nch in range(2):
                            ps = gen_ps.tile([128, 512], F32, tag="gen")
                            for i in range(4):
                                nc.tensor.matmul(
                                    ps[:],
                                    lhs_t[p][i][:, 128 * st : 128 * (st + 1)],
                                    wo_t[2 * i + p][:, 512 * nch : 512 * (nch + 1)],
                                    start=(i == 0),
                                    stop=(i == 3),
                                )
                            osl = slice(512 * nch, 512 * (nch + 1))
                            if p == 0:
                                nc.vector.tensor_add(osb[st][:, osl], ps[:], bo_bc[:, osl])
                            else:
                                nc.vector.tensor_add(osb[st][:, osl], ps[:], osb[st][:, osl])
                        if p == 1:
                            nc.sync.dma_start(out[128 * st : 128 * (st + 1), :], osb[st][:])

                emit_a2a_trigger(0)
                attn_qblock(1, 0)
                attn_qblock(1, 1)
                attn_qblock(1, 2)
                attn_qblock(1, 3)
                # keep the post-attention work out of the attention pipeline:
                # the scheduler otherwise hoists Wo work (gated on the slow
                # collective readback) ahead of attention matmuls
                tc.strict_bb_all_engine_barrier()
                emit_a2a_trigger(1)
                emit_readback(0)
                emit_wo(0)
                emit_readback(1)
                emit_wo(1)

    nc.finalize()
    return nc


def _prep_in_maps(q, k, v, W_q, b_q, W_k, b_k, W_v, b_v, W_o, b_o):
    def qpmajor(x):  # [S, D] -> [4, 128, 8*512]: [q, p, 512*t + c] = xT[128t+p, 512q+c]
        xt = x.T.astype(BF)  # [D, S]
        return np.ascontiguousarray(
            xt.reshape(8, 128, 4, 512).transpose(2, 1, 0, 3).reshape(4, 128, 4096)
        )

    xT = [(qpmajor(q[b]), qpmajor(k[b]), qpmajor(v[b])) for b in range(B)]
    wo_h = np.ascontiguousarray(W_o.astype(BF))
    bo_h = np.ascontiguousarray(b_o.reshape(1, D).astype(np.float32))
    i = np.arange(128)
    tri_h = np.ascontiguousarray((i[:, None] <= i[None, :]).astype(BF))

    def pmajor_w(w):  # [1024, 256] -> [128, 8*256]: [p, 256t + c] = w[128t+p, c]
        return np.ascontiguousarray(
            w.astype(BF).reshape(8, 128, DPC).transpose(1, 0, 2).reshape(128, 8 * DPC)
        )

    in_maps = []
    for c in range(N_CORES):
        b, hg = c // 4, c % 4
        csl = slice(DPC * hg, DPC * (hg + 1))
        in_maps.append(
            {
                "xq": xT[b][0],
                "xk": xT[b][1],
                "xv": xT[b][2],
                "wq": pmajor_w(W_q[:, csl]),
                "wk": pmajor_w(W_k[:, csl]),
                "wv": pmajor_w(W_v[:, csl]),
                "wo": wo_h,
                "bq": np.ascontiguousarray(
                    b_q[csl].reshape(2, 128).T.astype(np.float32)
                ),
                "bk": np.ascontiguousarray(
                    b_k[csl].reshape(2, 128).T.astype(np.float32)
                ),
                "bv": np.ascontiguousarray(b_v[csl].reshape(1, DPC).astype(np.float32)),
                "bo": bo_h,
                "tri": tri_h,
                "qoff": np.array([[b]], dtype=np.int32),
            }
        )
    return in_maps


def kernel(q, k, v, mask, W_q, b_q, W_k, b_k, W_v, b_v, W_o, b_o, **run_kwargs):
    q, k, v = (np.asarray(t, np.float32) for t in (q, k, v))
    in_maps = _prep_in_maps(
        q, k, v,
        np.asarray(W_q, np.float32), np.asarray(b_q, np.float32),
        np.asarray(W_k, np.float32), np.asarray(b_k, np.float32),
        np.asarray(W_v, np.float32), np.asarray(b_v, np.float32),
        np.asarray(W_o, np.float32), np.asarray(b_o, np.float32),
    )
    if "nc" not in _CACHED:
        _CACHED["nc"] = build_nc()
    res = run_bass_kernel_spmd(
        _CACHED["nc"], in_maps, core_ids=list(range(N_CORES)), **run_kwargs
    )
    _CACHED["last_result"] = res
    full = np.empty((B, S, D), np.float32)
    for c in range(N_CORES):
        b, hg = c // 4, c % 4
        full[b, 512 * hg : 512 * (hg + 1), :] = res.results[c]["out"]
    return full


if __name__ == "__main__":
    build_nc()
    print("build ok")


# revision 11
# speedup vs baseline: 1.1000x; 1.0212x over previous
"""MultiHeadAttention forward on 8 Trainium2 NeuronCores (v2).

Problem: B=2, S=2048, D_MODEL=1024, H=16 heads, d_k=64, causal mask.

Sharding v2: core c -> (batch b = c//4, head-group hg = c%4). Each core
computes attention for heads {4hg..4hg+3} of batch b, so it only loads
batch b's activations (12 MB instead of 24 MB per core).

 - Projections: Q^T,K^T per head-pair p in {0,1} as [128, S] tiles
   (rows: head 2p d_k 0-63, head 2p+1 64-127); V as packed [128, 130]
   tiles per 128-kpos block (cols 0:64 head even | col 64 ones |
   65:129 head odd | col 129 ones) -- the ones columns produce softmax
   denominators inside the attn@V matmuls.
 - Attention per pair, q-blocks of 512, kt blocks of 128 kpos.
   ScoresT[kpos, q] via two row-group-concurrent matmuls (K=64 each).
   exp on ScalarE (scale 1/8 folded). Diagonal kt tiles restrict all
   work (scores / exp / attnV) to the live columns [128t, 512) and
   multiply only the [128,128] triangle block by a causal mask.
 - Normalization: evacuate av psum [65, 1024], reciprocal_approx_fast
   on the denominator row, gpsimd partition-broadcast, multiply into
   AFT (attn_flatT, bf16).
 - Output: per head-pair AllToAll over the 4 cores of the same batch
   redistributes AFT so core j gets d-rows {256i+128p} for its q-slice
   [512j, 512j+512). Pair-0's collective + half of the W_o matmul
   overlap pair-1's attention. out = lhs.T @ W_o + b_o.

Matmuls in bf16 (host casts); fp32 accumulation in PSUM.
"""

import sys

import numpy as np

sys.path.insert(0, "/opt/trn_rl_repo")

import ml_dtypes  # noqa: E402

import concourse.bacc as bacc  # noqa: E402
import concourse.mybir as mybir  # noqa: E402
import concourse.tile as tile  # noqa: E402
from concourse.bass_utils import run_bass_kernel_spmd  # noqa: E402

F32 = mybir.dt.float32
BF16 = mybir.dt.bfloat16
BF = ml_dtypes.bfloat16

B, S, D, H, DK = 2, 2048, 1024, 16, 64
N_CORES = 8
HPC = 4  # heads per core
DPC = HPC * DK  # 256 d_model cols per core
NKT = S // 128  # 16 kpos tiles
NQB = S // 512  # 4 q blocks

_CACHED = {}


def build_nc():
    nc = bacc.Bacc(num_devices=N_CORES)

    # ---- I/O (per core: batch b = c//4, cols 256*hg..) ----
    # x stored quarter-major, p-major within quarter: [4, 128, 8*512]
    # (element [q, p, 512*t + c] = x[b].T[128*t + p, 512*q + c])
    xq = nc.dram_tensor("xq", [4, 128, 4096], BF16, kind="ExternalInput")
    xk = nc.dram_tensor("xk", [4, 128, 4096], BF16, kind="ExternalInput")
    xv = nc.dram_tensor("xv", [4, 128, 4096], BF16, kind="ExternalInput")
    # w p-major: [128, 8*256], element [p, 256*t + c] = W[128*t + p, c]
    wq = nc.dram_tensor("wq", [128, 8 * DPC], BF16, kind="ExternalInput")
    wk = nc.dram_tensor("wk", [128, 8 * DPC], BF16, kind="ExternalInput")
    wv = nc.dram_tensor("wv", [128, 8 * DPC], BF16, kind="ExternalInput")
    wo = nc.dram_tensor("wo", [D, D], BF16, kind="ExternalInput")
    bq = nc.dram_tensor("bq", [128, 2], F32, kind="ExternalInput")
    bk = nc.dram_tensor("bk", [128, 2], F32, kind="ExternalInput")
    bv = nc.dram_tensor("bv", [1, DPC], F32, kind="ExternalInput")
    bo = nc.dram_tensor("bo", [1, D], F32, kind="ExternalInput")
    tri = nc.dram_tensor("tri", [128, 128], BF16, kind="ExternalInput")
    qoff = nc.dram_tensor("qoff", [1, 1], mybir.dt.int32, kind="ExternalInput")
    out = nc.dram_tensor("out", [512, D], F32, kind="ExternalOutput")

    with tile.TileContext(nc) as tc:
        with (
            tc.tile_pool(name="wtiles", bufs=1) as w_pool,
            tc.tile_pool(name="persist", bufs=1) as persist,
            tc.tile_pool(name="exp", bufs=4) as exp_pool,
            tc.tile_pool(name="small", bufs=2) as small_pool,
            tc.tile_pool(name="gen_ps", bufs=2, space="PSUM") as gen_ps,
            tc.tile_pool(name="score_ps", bufs=2, space="PSUM") as score_ps,
            tc.tile_pool(name="av_ps", bufs=1, space="PSUM") as av_ps,
            tc.tile_pool(name="dram", bufs=1, space="DRAM") as dram,
        ):
            # ---- persistent SBUF tensors ----
            QT = [persist.tile([128, S], BF16, tag=f"QT{p}", name=f"QT{p}") for p in range(2)]
            KT = [persist.tile([128, S], BF16, tag=f"KT{p}", name=f"KT{p}") for p in range(2)]
            VP = [
                [persist.tile([128, 130], BF16, tag=f"VP{p}_{i}", name=f"VP{p}_{i}") for i in range(NKT)]
                for p in range(2)
            ]
            AFT = [persist.tile([128, S], BF16, tag=f"AFT{p}", name=f"AFT{p}") for p in range(2)]
            # packed weight tiles: [128, 8, 256] (d-tile t at [:, t, :]); one DMA
            # each, first in the queue so projections can start ASAP
            wq_a = w_pool.tile([128, 8, DPC], BF16, tag="wq_a")
            wk_a = w_pool.tile([128, 8, DPC], BF16, tag="wk_a")
            wv_a = w_pool.tile([128, 8, DPC], BF16, tag="wv_a")
            nc.scalar.dma_start(wk_a[:], wk[:])
            nc.scalar.dma_start(wq_a[:], wq[:])
            nc.scalar.dma_start(wv_a[:], wv[:])

            tri_t = persist.tile([128, 128], BF16, tag="tri")
            nc.scalar.dma_start(tri_t[:], tri[:])

            bq_t = persist.tile([128, 2], F32, tag="bq")
            bk_t = persist.tile([128, 2], F32, tag="bk")
            nc.scalar.dma_start(bq_t[:], bq[:])
            nc.scalar.dma_start(bk_t[:], bk[:])
            bv_bc = persist.tile([128, DPC], F32, tag="bvbc")
            nc.scalar.dma_start(bv_bc[:], bv[:].partition_broadcast(128))
            bo_bc = persist.tile([128, D], F32, tag="bobc")
            wq_t = [wq_a[:, d, :] for d in range(8)]
            wk_t = [wk_a[:, d, :] for d in range(8)]
            wv_t = [wv_a[:, d, :] for d in range(8)]
            wo_t = [w_pool.tile([128, D], BF16, tag=f"wo{d}", name=f"wo{d}") for d in range(8)]

            # ones columns of the packed V tiles (written once)
            for p in range(2):
                for i in range(NKT):
                    ones_ap = VP[p][i][:, 0:130].rearrange("a (h c) -> a h c", h=2)[:, :, 64:65]
                    nc.gpsimd.memset(ones_ap, 1.0)

            # ---- helpers ----
            def proj_qk_sc(p, sc, xa, wt, bias_t, dst):
                ps = gen_ps.tile([128, 512], F32, tag="gen", name="gen")
                for d in range(8):
                    nc.tensor.matmul(
                        ps[:],
                        wt[d][:, 128 * p : 128 * (p + 1)],
                        xa[:, sc, d, :],
                        start=(d == 0),
                        stop=(d == 7),
                    )
                nc.vector.tensor_scalar_add(
                    dst[:, 512 * sc : 512 * (sc + 1)], ps[:], bias_t[:, p : p + 1]
                )

            def proj_qk(p, xa, wt, bias_t, dst):
                for sc in range(4):
                    proj_qk_sc(p, sc, xa, wt, bias_t, dst)

            def proj_v_sc(sc, xa):
                for ss in range(4 * sc, 4 * sc + 4):
                    k = ss % 4
                    ps = gen_ps.tile([128, 512], F32, tag="gen", name="gen")
                    for d in range(8):
                        nc.tensor.matmul(
                            ps[:, 0:DPC],
                            xa[:, sc, d, 128 * k : 128 * (k + 1)],
                            wv_t[d],
                            start=(d == 0),
                            stop=(d == 7),
                        )
                    for p in range(2):
                        dst = VP[p][ss][:, 0:130].rearrange("a (h c) -> a h c", h=2)[:, :, 0:64]
                        src = ps[:, 128 * p : 128 * (p + 1)].rearrange("a (h c) -> a h c", h=2)
                        bsl = bv_bc[:, 128 * p : 128 * (p + 1)].rearrange("a (h c) -> a h c", h=2)
                        nc.vector.tensor_add(dst, src, bsl)

            def attn_qblock(p, qb):
                qcol = 512 * qb
                n_kt = 4 * qb + 4
                av = av_ps.tile([65, 1024], F32, tag="av")

                def emit_scores(kt):
                    t = kt - 4 * qb
                    c0 = 128 * t if t >= 0 else 0
                    ps = score_ps.tile([128, 1024], F32, tag="sc", name="sc")
                    ksl = slice(128 * kt, 128 * (kt + 1))
                    for h in range(2):
                        nc.tensor.matmul(
                            ps[:, 512 * h + c0 : 512 * (h + 1)],
                            KT[p][64 * h : 64 * (h + 1), ksl],
                            QT[p][64 * h : 64 * (h + 1), qcol + c0 : qcol + 512],
                            start=True,
                            stop=True,
                        )
                    return ps

                ps_cur = emit_scores(0)
                for kt in range(n_kt):
                    t = kt - 4 * qb
                    c0 = 128 * t if t >= 0 else 0
                    et = exp_pool.tile([128, 1024], BF16, tag="et")
                    if c0 == 0:
                        nc.scalar.activation(
                            et[:], ps_cur[:], mybir.ActivationFunctionType.Exp, scale=0.125
                        )
                    else:
                        e3 = et[:, 0:1024].rearrange("a (h q) -> a h q", h=2)[:, :, c0:512]
                        p3 = ps_cur[:, 0:1024].rearrange("a (h q) -> a h q", h=2)[:, :, c0:512]
                        nc.scalar.activation(
                            e3, p3, mybir.ActivationFunctionType.Exp, scale=0.125
                        )
                    if kt + 1 < n_kt:
                        ps_cur = emit_scores(kt + 1)
                    if t >= 0:
                        for h in range(2):
                            dsl = slice(512 * h + c0, 512 * h + c0 + 128)
                            nc.vector.tensor_mul(et[:, dsl], et[:, dsl], tri_t[:])
                    for h in range(2):
                        nc.tensor.matmul(
                            av[:, 512 * h + c0 : 512 * (h + 1)],
                            VP[p][kt][:, 65 * h : 65 * (h + 1)],
                            et[:, 512 * h + c0 : 512 * (h + 1)],
                            start=(kt == 0),
                            stop=(kt == n_kt - 1),
                        )
                # normalize: evacuate psum, 1/denom, broadcast, scale into AFT
                avs = small_pool.tile([64, 1024], F32, tag="avs", name="avs")
                nc.vector.tensor_copy(avs[:], av[0:64, :])
                dn = small_pool.tile([1, 1024], F32, tag="dn")
                nc.vector.tensor_copy(dn[:], av[64:65, :])
                rcp = small_pool.tile([1, 1024], F32, tag="rcp")
                nc.vector.reciprocal_approx_fast(rcp[:], dn[:])
                rbc = small_pool.tile([64, 1024], F32, tag="rbc")
                nc.gpsimd.partition_broadcast(rbc[:], rcp[:])
                for h in range(2):
                    nc.vector.tensor_mul(
                        AFT[p][64 * h : 64 * (h + 1), qcol : qcol + 512],
                        avs[0:64, 512 * h : 512 * (h + 1)],
                        rbc[:, 512 * h : 512 * (h + 1)],
                    )
                # stage this q-block's columns for the AllToAll (dests qb, qb+4)
                for j in (qb, qb + 4):
                    nc.sync.dma_start(
                        a2a_in[p][128 * j : 128 * (j + 1), :],
                        AFT[p][:, qcol : qcol + 512],
                    )

            a2a_in = [dram.tile([1024, 512], BF16, tag=f"a2a_in{p}", name=f"a2a_in{p}") for p in range(2)]
            a2a_out = [dram.tile([2, 512, 512], BF16, tag=f"a2a_out{p}", name=f"a2a_out{p}") for p in range(2)]

            # ---- load x, project, attention pair 0 (pair-1 proj interleaved) ----
            with (
                tc.tile_pool(name="xtq", bufs=1) as xq_pool,
                tc.tile_pool(name="xtk", bufs=1) as xk_pool,
                tc.tile_pool(name="xtv", bufs=1) as xv_pool,
            ):
                xk_a = xk_pool.tile([128, 4, 8, 512], BF16, tag="xk_a")
                xv_a = xv_pool.tile([128, 4, 8, 512], BF16, tag="xv_a")
                xq_a = xq_pool.tile([128, 4, 8, 512], BF16, tag="xq_a")
                # quarter loads: one fully-contiguous [128 x 8KB] DMA each, so
                # pair-0 projections and attention q-block 0 start early
                for sc in range(4):
                    for xa_, xd in ((xk_a, xk), (xq_a, xq), (xv_a, xv)):
                        nc.sync.dma_start(xa_[:, sc, :, :], xd[sc, :, :])
                    proj_qk_sc(0, sc, xk_a, wk_t, bk_t, KT[0])
                    proj_qk_sc(0, sc, xq_a, wq_t, bq_t, QT[0])
                    proj_v_sc(sc, xv_a)

                attn_qblock(0, 0)
                # W_o / b_o loads late: needed only by the post-attention Wo
                for d in range(8):
                    nc.sync.dma_start(wo_t[d][:], wo[128 * d : 128 * (d + 1), :])
                nc.sync.dma_start(bo_bc[:], bo[:].partition_broadcast(128))
                proj_qk(1, xk_a, wk_t, bk_t, KT[1])
                attn_qblock(0, 1)
                proj_qk(1, xq_a, wq_t, bq_t, QT[1])
                attn_qblock(0, 2)
                attn_qblock(0, 3)

            with tc.tile_pool(name="late", bufs=1) as late:
                lhs_t = [
                    [late.tile([128, 512], BF16, tag=f"lhs{p}_{i}", name=f"lhs{p}_{i}") for i in range(4)]
                    for p in range(2)
                ]
                osb = [late.tile([128, D], F32, tag=f"osb{st}", name=f"osb{st}") for st in range(4)]

                # my batch-group (core // 4), from a per-core input
                import concourse.bass as bass_mod
                gsel_reg = nc.alloc_registers("gsel_reg")
                nc.regs_load(gsel_reg, qoff[0:1, 0:1])
                gsel_s = nc.snap(gsel_reg, donate=True, min_val=0, max_val=1)

                def emit_a2a_trigger(p):
                    nc.gpsimd.collective_compute(
                        "AllToAll",
                        mybir.AluOpType.bypass,
                        replica_groups=[list(range(8))],
                        ins=[a2a_in[p][:]],
                        outs=[a2a_out[p][:]],
                    )

                def emit_readback(p):
                    for i in range(4):
                        nc.sync.dma_start(
                            lhs_t[p][i][:],
                            a2a_out[p][bass_mod.ds(gsel_s, 1), 128 * i : 128 * (i + 1), :],
                        )

                def emit_wo(p):
                    for st in range(4):
                        for nch in range(2):
                            ps = gen_ps.tile([128, 512], F32, tag="gen")
                            for i in range(4):
                                nc.tensor.matmul(
                                    ps[:],
                                    lhs_t[p][i][:, 128 * st : 128 * (st + 1)],
                                    wo_t[2 * i + p][:, 512 * nch : 512 * (nch + 1)],
                                    start=(i == 0),
                                    stop=(i == 3),
                                )
                            osl = slice(512 * nch, 512 * (nch + 1))
                            if p == 0:
                                nc.vector.tensor_add(osb[st][:, osl], ps[:], bo_bc[:, osl])
                            else:
                                nc.vector.tensor_add(osb[st][:, osl], ps[:], osb[st][:, osl])
                        if p == 1:
                            nc.sync.dma_start(out[128 * st : 128 * (st + 1), :], osb[st][:])

                emit_a2a_trigger(0)
                attn_qblock(1, 0)
                attn_qblock(1, 1)
                attn_qblock(1, 2)
                attn_qblock(1, 3)
                # keep the post-attention work out of the attention pipeline:
                # the scheduler otherwise hoists Wo work (gated on the slow
                # collective readback) ahead of attention matmuls
                tc.strict_bb_all_engine_barrier()
                emit_a2a_trigger(1)
                emit_readback(0)
                emit_wo(0)
                emit_readback(1)
                emit_wo(1)

    nc.finalize()
    return nc


def _prep_in_maps(q, k, v, W_q, b_q, W_k, b_k, W_v, b_v, W_o, b_o):
    def qpmajor(x):  # [S, D] -> [4, 128, 8*512]: [q, p, 512*t + c] = xT[128t+p, 512q+c]
        xt = x.T.astype(BF)  # [D, S]
        return np.ascontiguousarray(
            xt.reshape(8, 128, 4, 512).transpose(2, 1, 0, 3).reshape(4, 128, 4096)
        )

    xT = [(qpmajor(q[b]), qpmajor(k[b]), qpmajor(v[b])) for b in range(B)]
    wo_h = np.ascontiguousarray(W_o.astype(BF))
    bo_h = np.ascontiguousarray(b_o.reshape(1, D).astype(np.float32))
    i = np.arange(128)
    tri_h = np.ascontiguousarray((i[:, None] <= i[None, :]).astype(BF))

    def pmajor_w(w):  # [1024, 256] -> [128, 8*256]: [p, 256t + c] = w[128t+p, c]
        return np.ascontiguousarray(
            w.astype(BF).reshape(8, 128, DPC).transpose(1, 0, 2).reshape(128, 8 * DPC)
        )

    in_maps = []
    for c in range(N_CORES):
        b, hg = c // 4, c % 4
        csl = slice(DPC * hg, DPC * (hg + 1))
        in_maps.append(
            {
                "xq": xT[b][0],
                "xk": xT[b][1],
                "xv": xT[b][2],
                "wq": pmajor_w(W_q[:, csl]),
                "wk": pmajor_w(W_k[:, csl]),
                "wv": pmajor_w(W_v[:, csl]),
                "wo": wo_h,
                "bq": np.ascontiguousarray(
                    b_q[csl].reshape(2, 128).T.astype(np.float32)
                ),
                "bk": np.ascontiguousarray(
                    b_k[csl].reshape(2, 128).T.astype(np.float32)
                ),
                "bv": np.ascontiguousarray(b_v[csl].reshape(1, DPC).astype(np.float32)),
                "bo": bo_h,
                "tri": tri_h,
                "qoff": np.array([[b]], dtype=np.int32),
            }
        )
    return in_maps


def kernel(q, k, v, mask, W_q, b_q, W_k, b_k, W_v, b_v, W_o, b_o, **run_kwargs):
    q, k, v = (np.asarray(t, np.float32) for t in (q, k, v))
    in_maps = _prep_in_maps(
        q, k, v,
        np.asarray(W_q, np.float32), np.asarray(b_q, np.float32),
        np.asarray(W_k, np.float32), np.asarray(b_k, np.float32),
        np.asarray(W_v, np.float32), np.asarray(b_v, np.float32),
        np.asarray(W_o, np.float32), np.asarray(b_o, np.float32),
    )
    if "nc" not in _CACHED:
        _CACHED["nc"] = build_nc()
    res = run_bass_kernel_spmd(
        _CACHED["nc"], in_maps, core_ids=list(range(N_CORES)), **run_kwargs
    )
    _CACHED["last_result"] = res
    full = np.empty((B, S, D), np.float32)
    for c in range(N_CORES):
        b, hg = c // 4, c % 4
        full[b, 512 * hg : 512 * (hg + 1), :] = res.results[c]["out"]
    return full


if __name__ == "__main__":
    build_nc()
    print("build ok")
